# revision 25
# baseline (speedup 1.0000x reference)
"""Chamfer-distance (CDLoss) Trainium2 kernel.

Strategy: data-parallel over the 16 point clouds -> 2 clouds per NeuronCore.
Each core computes, per cloud, the full 4096x4096 squared-distance matrix in
128x4096 chunks via a single K=13 bf16 matmul (hi/lo split of
[x, |x|^2, 1] x [-2y, 1, |y|^2] for fp32-class accuracy at bf16 speed),
then fuses row-min (fold tree on VectorE) and col-min (elementwise min
accumulate) on-chip.  The per-cloud row-mins [128,32] and the col-min
accumulator [128,4096] are DMA'd out; the host finishes the tiny
partition-axis min + means and averages over clouds.
"""

import os
import sys

import numpy as np

sys.path.insert(0, "/opt/trn_rl_repo")

B = 16
N = 4096
D = 3
NCORES = 8
CPC = B // NCORES  # clouds per core
K = 13  # contraction rows after hi/lo bf16 split
NCHUNK = N // 128  # 32 row-chunks per cloud

# Populated by the most recent kernel() call when tracing is enabled.
LAST_EXEC_NS = None
TRACE = bool(int(os.environ.get("CD_TRACE", "0")))

_CACHE = {}


def _install_profile_shim():
    """This container's antenv package lacks axon_hooks, so bass_utils can't
    NTFF-profile under axon.  Provide the module and install the ctypes hook
    against the axon PJRT plugin (degrades silently if unavailable)."""
    import types

    if "antenv.axon_hooks" in sys.modules:
        return
    try:
        import antenv
        from trn_agent_boot.trn_boot import _ntff_profile_via_ctypes

        m = types.ModuleType("antenv.axon_hooks")
        _h = {"hook": None}
        m.set_axon_ntff_profile_hook = lambda h: _h.__setitem__("hook", h)
        m.get_axon_ntff_profile_hook = lambda: _h["hook"]
        sys.modules["antenv.axon_hooks"] = m
        antenv.axon_hooks = m
        m.set_axon_ntff_profile_hook(
            _ntff_profile_via_ctypes("/opt/axon/libaxon_pjrt.so")
        )
    except Exception:
        pass


def _patch_tail_drain():
    """The walrus build in this container accepts only ONE semaphore wait per
    instruction, but TileContext's kernel-tail drain aggregates a wait per
    live processor onto a single SP Drain.  Split them: one single-wait SP
    NOP per extra processor, chained in front of the drain."""
    from concourse import mybir
    from concourse import tile as tile_mod
    from concourse.vector_clock import ScopedClock

    if getattr(tile_mod.TileContext, "_cd_tail_patched", False):
        return

    def _drain_and_barrier(self, tick_clock, wait_clock):
        drain_inst = self.nc.sync.drain()
        wait_clock.add_sem_waits(
            drain_inst.ins, ScopedClock({None: tick_clock.global_clock})
        )
        si = drain_inst.ins.sync_info
        waits = list(si.on_wait) if si is not None and si.on_wait else []
        if len(waits) > 1:
            drain_inst.ins.sync_info = mybir.SyncInfo(
                on_wait=[waits[-1]], on_update=list(si.on_update or [])
            )
            bb = self.nc.cur_bb.bb
            insts = bb.instructions
            idx = insts.index(drain_inst.ins)
            for j, w in enumerate(waits[:-1]):
                nop = self.nc.sync.nop()
                nop.ins.sync_info = mybir.SyncInfo(on_wait=[w], on_update=[])
                insts.remove(nop.ins)
                insts.insert(idx + j, nop.ins)

        self.nc.all_engine_barrier()
        assert self.sems is not None
        popped = self.nc._tile_sem_poison_stack.pop()
        assert popped is self._sem_poison
        self.nc.clear_and_free_semaphores(list(self.sems.allocated().values()))
        self.nc.all_engine_barrier()

    tile_mod.TileContext._drain_and_barrier = _drain_and_barrier
    tile_mod.TileContext._cd_tail_patched = True


def _build_bass():
    from concourse import bass, mybir
    from concourse.tile import TileContext, add_dep_helper

    _patch_tail_drain()

    bf16 = mybir.dt.bfloat16
    f16 = mybir.dt.float16
    f32 = mybir.dt.float32
    MIN = mybir.AluOpType.min

    RES_W = N + NCHUNK * 256  # per-cloud output width: colacc || rowmin f4 blocks

    nc = bass.Bass()
    # Packed input: inp[k, c, j, n] with j=0 -> Xp row, j=1 -> Yp row.
    inp = nc.declare_dram_parameter("inp", [K, CPC, 2, N], bf16, isOutput=False)
    outp = nc.declare_dram_parameter("out", [128, CPC * RES_W], f16, isOutput=True)

    with TileContext(nc) as tc:
        with (
            tc.tile_pool(name="const", bufs=1) as cpool,
            tc.tile_pool(name="work", bufs=3) as wpool,
            tc.tile_pool(name="psum", bufs=2, space="PSUM") as ppool,
            tc.tile_pool(name="accs", bufs=1) as apool,
        ):
            # Scratch sinks for the wait-absorber copies below; one fresh
            # element per chunk so the absorbers never pick up WAW deps.
            scr_a = apool.tile([1, CPC * NCHUNK], f16, tag="scr_a")
            scr_b = apool.tile([1, CPC * NCHUNK], f16, tag="scr_b")
            # Single big input tile [K, CPC*2*N] and single result tile so the
            # kernel needs only 2 DMA instructions (the final Drain's wait
            # budget caps how many DMA queues may be live).
            xy_sb = cpool.tile([K, CPC * 2 * N], bf16, tag="xy")
            nc.sync.dma_start(out=xy_sb, in_=inp[:])
            res = apool.tile([128, CPC * RES_W], f16, tag="res")

            prev_stage = None
            prev_colacc = None
            for c in range(CPC):
                xp_sb = xy_sb[:, (2 * c) * N : (2 * c + 1) * N]
                yp_sb = xy_sb[:, (2 * c + 1) * N : (2 * c + 2) * N]

                colacc = res[:, c * RES_W : c * RES_W + N]
                rowaccs = res[:, c * RES_W + N : (c + 1) * RES_W]

                # PE wait-absorber: a throwaway weight load that carries the
                # yp DMA wait, keeping the first real matmul of this cloud
                # within the single-wait budget of the MM instruction.
                nc.tensor.ldweights(weights=yp_sb[:, 0:1])

                for ci in range(NCHUNK):
                    stage = wpool.tile([128, N], f16, tag="stage")
                    lhsT = xp_sb[:, ci * 128 : (ci + 1) * 128]
                    # Wait-absorbers: ScalarE instructions may carry only ONE
                    # semaphore wait (walrus S3D3_AC limit).  The first cast
                    # into a reused stage slot would need {PE, DVE, ACT}
                    # waits; these two single-wait copies advance ScalarE's
                    # observed DVE / ACT ticks first so the casts only wait
                    # on PE.
                    idx = c * NCHUNK + ci
                    absorbers = []
                    dve_src = colacc if ci > 0 else prev_colacc
                    if dve_src is not None:
                        absorbers.append(
                            nc.scalar.copy(
                                out=scr_b[0:1, idx : idx + 1], in_=dve_src[0:1, 0:1]
                            )
                        )
                    if prev_stage is not None:
                        absorbers.append(
                            nc.scalar.copy(
                                out=scr_a[0:1, idx : idx + 1],
                                in_=prev_stage[0:1, N - 1 : N],
                            )
                        )
                    for half in range(2):
                        ps = ppool.tile([128, 2048], f32, tag="ps")
                        ldw = None
                        if prev_stage is not None:
                            # PE wait-absorber: carries the ACT tick of the
                            # cast that last read this (reused) PSUM slot, so
                            # the first matmul below keeps a single wait.
                            ldw = nc.tensor.ldweights(
                                weights=prev_stage[0:1, half * 2048 : half * 2048 + 1]
                            )
                        for mb in range(4):
                            m0 = half * 2048 + mb * 512
                            mm = nc.tensor.matmul(
                                out=ps[:, mb * 512 : (mb + 1) * 512],
                                lhsT=lhsT,
                                rhs=yp_sb[:, m0 : m0 + 512],
                                start=True,
                                stop=True,
                            )
                            if mb == 0 and ldw is not None:
                                add_dep_helper(
                                    mm.ins, ldw.ins, sync=False, reason="ldw order"
                                )
                        # fp32 PSUM -> fp16 SBUF cast on ScalarE, 2048 wide
                        cast = nc.scalar.copy(
                            out=stage[:, half * 2048 : (half + 1) * 2048], in_=ps
                        )
                        for ab in absorbers:
                            add_dep_helper(
                                cast.ins, ab.ins, sync=False, reason="absorber order"
                            )
                    prev_stage = stage

                    # col-min accumulate first (VectorE, fp16 2x mode) so the
                    # fold tree's DVE deps stay in program order behind it.
                    if ci == 0:
                        cm = nc.vector.tensor_copy(out=colacc, in_=stage)
                    else:
                        cm = nc.vector.tensor_tensor(
                            out=colacc, in0=stage, in1=colacc, op=MIN
                        )

                    # row-min fold tree (VectorE, fp16 2x mode); the final
                    # 256-wide block lands in res and the host finishes it.
                    f1 = wpool.tile([128, 2048], f16, tag="f1")
                    fold1 = nc.vector.tensor_tensor(
                        out=f1, in0=stage[:, :2048], in1=stage[:, 2048:], op=MIN
                    )
                    add_dep_helper(
                        fold1.ins, cm.ins, sync=False, reason="colmin first"
                    )
                    f2 = wpool.tile([128, 1024], f16, tag="f2")
                    nc.vector.tensor_tensor(
                        out=f2, in0=f1[:, :1024], in1=f1[:, 1024:], op=MIN
                    )
                    f3 = wpool.tile([128, 512], f16, tag="f3")
                    nc.vector.tensor_tensor(
                        out=f3, in0=f2[:, :512], in1=f2[:, 512:], op=MIN
                    )
                    nc.vector.tensor_tensor(
                        out=rowaccs[:, ci * 256 : (ci + 1) * 256],
                        in0=f3[:, :256],
                        in1=f3[:, 256:],
                        op=MIN,
                    )

                prev_colacc = colacc
                nc.sync.dma_start(
                    out=outp[:, c * RES_W : (c + 1) * RES_W],
                    in_=res[:, c * RES_W : (c + 1) * RES_W],
                )

    return nc


def _get_nc():
    if "nc" not in _CACHE:
        _CACHE["nc"] = _build_bass()
    return _CACHE["nc"]


def _to_dense(x, batch):
    """Replicate PyG to_dense_batch + jax scatter-drop semantics."""
    x = np.asarray(x, np.float32)
    batch = np.asarray(batch).astype(np.int64)
    counts = np.bincount(batch, minlength=B)[:B]
    offsets = np.concatenate([[0], np.cumsum(counts)[:-1]])
    pos = np.arange(batch.shape[0], dtype=np.int64) - offsets[batch]
    dense = np.zeros((B, N, D), np.float32)
    valid = (pos >= 0) & (pos < N) & (batch >= 0) & (batch < B)
    dense[batch[valid], pos[valid]] = x[valid]
    return dense


def _hi_lo(v):
    import ml_dtypes

    hi = v.astype(np.float32).astype(ml_dtypes.bfloat16)
    lo = (v.astype(np.float32) - hi.astype(np.float32)).astype(ml_dtypes.bfloat16)
    return hi, lo


def _make_operands(x, y):
    """x, y: [N, 3] fp32 for one cloud -> (XpT, YpT) [13, N] bf16."""
    import ml_dtypes

    xT = x.T.astype(np.float64)  # [3, N]
    yT = y.T.astype(np.float64)
    x2 = (xT * xT).sum(axis=0)  # [N]
    y2 = (yT * yT).sum(axis=0)
    y2m = -2.0 * yT  # [3, N]

    Xp = np.zeros((K, N), ml_dtypes.bfloat16)
    Yp = np.zeros((K, N), ml_dtypes.bfloat16)
    ones = np.ones((N,), ml_dtypes.bfloat16)
    for i in range(D):
        hx, lx = _hi_lo(xT[i])
        hy, ly = _hi_lo(y2m[i])
        Xp[3 * i + 0], Yp[3 * i + 0] = hx, hy
        Xp[3 * i + 1], Yp[3 * i + 1] = hx, ly
        Xp[3 * i + 2], Yp[3 * i + 2] = lx, hy
    hx2, lx2 = _hi_lo(x2)
    hy2, ly2 = _hi_lo(y2)
    Xp[9], Yp[9] = hx2, ones
    Xp[10], Yp[10] = lx2, ones
    Xp[11], Yp[11] = ones, hy2
    Xp[12], Yp[12] = ones, ly2
    return Xp, Yp


def kernel(pred, target, batch):
    global LAST_EXEC_NS
    from concourse.bass_utils import run_bass_kernel_spmd

    import ml_dtypes

    xd = _to_dense(pred, batch)  # [B, N, 3]
    yd = _to_dense(target, batch)

    RES_W = N + NCHUNK * 256
    in_maps = []
    for core in range(NCORES):
        inp = np.zeros((K, CPC, 2, N), ml_dtypes.bfloat16)
        for c in range(CPC):
            b = core * CPC + c
            Xp, Yp = _make_operands(xd[b], yd[b])
            inp[:, c, 0, :] = Xp
            inp[:, c, 1, :] = Yp
        in_maps.append({"inp": inp})

    if TRACE:
        _install_profile_shim()
    nc = _get_nc()
    res = run_bass_kernel_spmd(
        nc, in_maps, core_ids=list(range(NCORES)), trace=TRACE
    )
    LAST_EXEC_NS = res.exec_time_ns

    total = 0.0
    for core in range(NCORES):
        out = np.asarray(res.results[core]["out"], np.float64)  # [128, CPC*RES_W]
        for c in range(CPC):
            colacc = out[:, c * RES_W : c * RES_W + N]
            rowblk = out[:, c * RES_W + N : (c + 1) * RES_W]
            rowmins = rowblk.reshape(128, NCHUNK, 256).min(axis=2)
            cham_x = rowmins.mean()
            cham_y = colacc.min(axis=0).mean()
            total += cham_x + cham_y
    return np.float32(total / B)


# revision 26
# speedup vs baseline: 1.3756x; 1.3756x over previous
"""Chamfer-distance (CDLoss) Trainium2 kernel.

Strategy: data-parallel over the 16 point clouds -> 2 clouds per NeuronCore.
Each core computes, per cloud, the full 4096x4096 squared-distance matrix in
128x4096 chunks via a single K=13 bf16 matmul (hi/lo split of
[x, |x|^2, 1] x [-2y, 1, |y|^2] for fp32-class accuracy at bf16 speed),
then fuses row-min (fold tree on VectorE) and col-min (elementwise min
accumulate) on-chip.  The per-cloud row-mins [128,32] and the col-min
accumulator [128,4096] are DMA'd out; the host finishes the tiny
partition-axis min + means and averages over clouds.
"""

import os
import sys

import numpy as np

sys.path.insert(0, "/opt/trn_rl_repo")

B = 16
N = 4096
D = 3
NCORES = 8
CPC = B // NCORES  # clouds per core
K = 13  # contraction rows after hi/lo bf16 split
NCHUNK = N // 128  # 32 row-chunks per cloud

# Populated by the most recent kernel() call when tracing is enabled.
LAST_EXEC_NS = None
TRACE = bool(int(os.environ.get("CD_TRACE", "0")))

_CACHE = {}


def _install_profile_shim():
    """This container's antenv package lacks axon_hooks, so bass_utils can't
    NTFF-profile under axon.  Provide the module and install the ctypes hook
    against the axon PJRT plugin (degrades silently if unavailable)."""
    import types

    if "antenv.axon_hooks" in sys.modules:
        return
    try:
        import antenv
        from trn_agent_boot.trn_boot import _ntff_profile_via_ctypes

        m = types.ModuleType("antenv.axon_hooks")
        _h = {"hook": None}
        m.set_axon_ntff_profile_hook = lambda h: _h.__setitem__("hook", h)
        m.get_axon_ntff_profile_hook = lambda: _h["hook"]
        sys.modules["antenv.axon_hooks"] = m
        antenv.axon_hooks = m
        m.set_axon_ntff_profile_hook(
            _ntff_profile_via_ctypes("/opt/axon/libaxon_pjrt.so")
        )
    except Exception:
        pass


def _patch_tail_drain():
    """The walrus build in this container accepts only ONE semaphore wait per
    instruction, but TileContext's kernel-tail drain aggregates a wait per
    live processor onto a single SP Drain.  Split them: one single-wait SP
    NOP per extra processor, chained in front of the drain."""
    from concourse import mybir
    from concourse import tile as tile_mod
    from concourse.vector_clock import ScopedClock

    if getattr(tile_mod.TileContext, "_cd_tail_patched", False):
        return

    def _drain_and_barrier(self, tick_clock, wait_clock):
        drain_inst = self.nc.sync.drain()
        wait_clock.add_sem_waits(
            drain_inst.ins, ScopedClock({None: tick_clock.global_clock})
        )
        si = drain_inst.ins.sync_info
        waits = list(si.on_wait) if si is not None and si.on_wait else []
        if len(waits) > 1:
            drain_inst.ins.sync_info = mybir.SyncInfo(
                on_wait=[waits[-1]], on_update=list(si.on_update or [])
            )
            bb = self.nc.cur_bb.bb
            insts = bb.instructions
            idx = insts.index(drain_inst.ins)
            for j, w in enumerate(waits[:-1]):
                nop = self.nc.sync.nop()
                nop.ins.sync_info = mybir.SyncInfo(on_wait=[w], on_update=[])
                insts.remove(nop.ins)
                insts.insert(idx + j, nop.ins)

        self.nc.all_engine_barrier()
        assert self.sems is not None
        popped = self.nc._tile_sem_poison_stack.pop()
        assert popped is self._sem_poison
        self.nc.clear_and_free_semaphores(list(self.sems.allocated().values()))
        self.nc.all_engine_barrier()

    tile_mod.TileContext._drain_and_barrier = _drain_and_barrier
    tile_mod.TileContext._cd_tail_patched = True


def _build_bass():
    from concourse import bass, mybir
    from concourse.tile import TileContext, add_dep_helper

    _patch_tail_drain()

    bf16 = mybir.dt.bfloat16
    f16 = mybir.dt.float16
    f32 = mybir.dt.float32
    MIN = mybir.AluOpType.min

    RES_W = N + NCHUNK * 256  # per-cloud output width: colacc || rowmin f4 blocks

    nc = bass.Bass()
    # Packed input: inp[k, c, j, n] with j=0 -> Xp row, j=1 -> Yp row.
    inp = nc.declare_dram_parameter("inp", [K, CPC, 2, N], bf16, isOutput=False)
    outp = nc.declare_dram_parameter("out", [128, CPC * RES_W], f16, isOutput=True)

    with TileContext(nc) as tc:
        with (
            tc.tile_pool(name="const", bufs=1) as cpool,
            tc.tile_pool(name="work", bufs=3) as wpool,
            tc.tile_pool(name="psum", bufs=2, space="PSUM") as ppool,
            tc.tile_pool(name="accs", bufs=1) as apool,
        ):
            # Scratch sinks for the wait-absorber copies below; one fresh
            # element per chunk so the absorbers never pick up WAW deps.
            scr_a = apool.tile([1, CPC * NCHUNK], f16, tag="scr_a")
            scr_b = apool.tile([1, CPC * NCHUNK], f16, tag="scr_b")
            # Single big input tile [K, CPC*2*N] and single result tile so the
            # kernel needs only 2 DMA instructions (the final Drain's wait
            # budget caps how many DMA queues may be live).
            xy_sb = cpool.tile([K, CPC * 2 * N], bf16, tag="xy")
            nc.sync.dma_start(out=xy_sb, in_=inp[:])
            res = apool.tile([128, CPC * RES_W], f16, tag="res")

            prev_stage = None
            prev_colacc = None
            for c in range(CPC):
                xp_sb = xy_sb[:, (2 * c) * N : (2 * c + 1) * N]
                yp_sb = xy_sb[:, (2 * c + 1) * N : (2 * c + 2) * N]

                colacc = res[:, c * RES_W : c * RES_W + N]
                rowaccs = res[:, c * RES_W + N : (c + 1) * RES_W]

                # PE wait-absorber: a throwaway weight load that carries the
                # yp DMA wait, keeping the first real matmul of this cloud
                # within the single-wait budget of the MM instruction.
                nc.tensor.ldweights(weights=yp_sb[:, 0:1])

                for ci in range(NCHUNK):
                    stage = wpool.tile([128, N], f16, tag="stage")
                    lhsT = xp_sb[:, ci * 128 : (ci + 1) * 128]
                    # Wait-absorbers: ScalarE instructions may carry only ONE
                    # semaphore wait (walrus S3D3_AC limit).  The first cast
                    # into a reused stage slot would need {PE, DVE, ACT}
                    # waits; these two single-wait copies advance ScalarE's
                    # observed DVE / ACT ticks first so the casts only wait
                    # on PE.
                    idx = c * NCHUNK + ci
                    absorbers = []
                    # DVE-tick absorber: read the f4 row-block of the chunk
                    # whose stage slot is being reused (3 chunks back) — its
                    # f4 was the last DVE reader of that slot and completed
                    # long ago, so this wait never stalls.
                    gidx = idx - 3
                    if gidx >= 0:
                        cc, cci = divmod(gidx, NCHUNK)
                        src = res[0:1, cc * RES_W + N + cci * 256 :][0:1, 0:1]
                        absorbers.append(
                            nc.scalar.copy(out=scr_b[0:1, idx : idx + 1], in_=src)
                        )
                    if prev_stage is not None:
                        absorbers.append(
                            nc.scalar.copy(
                                out=scr_a[0:1, idx : idx + 1],
                                in_=prev_stage[0:1, N - 1 : N],
                            )
                        )
                    for half in range(2):
                        ps = ppool.tile([128, 2048], f32, tag="ps")
                        ldw = None
                        if prev_stage is not None:
                            # PE wait-absorber: carries the ACT tick of the
                            # cast that last read this (reused) PSUM slot, so
                            # the first matmul below keeps a single wait.
                            ldw = nc.tensor.ldweights(
                                weights=prev_stage[0:1, half * 2048 : half * 2048 + 1]
                            )
                        for mb in range(4):
                            m0 = half * 2048 + mb * 512
                            mm = nc.tensor.matmul(
                                out=ps[:, mb * 512 : (mb + 1) * 512],
                                lhsT=lhsT,
                                rhs=yp_sb[:, m0 : m0 + 512],
                                start=True,
                                stop=True,
                            )
                            if mb == 0 and ldw is not None:
                                add_dep_helper(
                                    mm.ins, ldw.ins, sync=False, reason="ldw order"
                                )
                        # fp32 PSUM -> fp16 SBUF cast on ScalarE, 2048 wide
                        cast = nc.scalar.copy(
                            out=stage[:, half * 2048 : (half + 1) * 2048], in_=ps
                        )
                        for ab in absorbers:
                            add_dep_helper(
                                cast.ins, ab.ins, sync=False, reason="absorber order"
                            )
                    prev_stage = stage

                    # col-min accumulate first (VectorE, fp16 2x mode) so the
                    # fold tree's DVE deps stay in program order behind it.
                    if ci == 0:
                        cm = nc.vector.tensor_copy(out=colacc, in_=stage)
                    else:
                        cm = nc.vector.tensor_tensor(
                            out=colacc, in0=stage, in1=colacc, op=MIN
                        )

                    # row-min fold tree (VectorE, fp16 2x mode); the final
                    # 256-wide block lands in res and the host finishes it.
                    f1 = wpool.tile([128, 2048], f16, tag="f1")
                    fold1 = nc.vector.tensor_tensor(
                        out=f1, in0=stage[:, :2048], in1=stage[:, 2048:], op=MIN
                    )
                    add_dep_helper(
                        fold1.ins, cm.ins, sync=False, reason="colmin first"
                    )
                    f2 = wpool.tile([128, 1024], f16, tag="f2")
                    nc.vector.tensor_tensor(
                        out=f2, in0=f1[:, :1024], in1=f1[:, 1024:], op=MIN
                    )
                    f3 = wpool.tile([128, 512], f16, tag="f3")
                    nc.vector.tensor_tensor(
                        out=f3, in0=f2[:, :512], in1=f2[:, 512:], op=MIN
                    )
                    nc.vector.tensor_tensor(
                        out=rowaccs[:, ci * 256 : (ci + 1) * 256],
                        in0=f3[:, :256],
                        in1=f3[:, 256:],
                        op=MIN,
                    )

                prev_colacc = colacc
                nc.sync.dma_start(
                    out=outp[:, c * RES_W : (c + 1) * RES_W],
                    in_=res[:, c * RES_W : (c + 1) * RES_W],
                )

    return nc


def _get_nc():
    if "nc" not in _CACHE:
        _CACHE["nc"] = _build_bass()
    return _CACHE["nc"]


def _to_dense(x, batch):
    """Replicate PyG to_dense_batch + jax scatter-drop semantics."""
    x = np.asarray(x, np.float32)
    batch = np.asarray(batch).astype(np.int64)
    counts = np.bincount(batch, minlength=B)[:B]
    offsets = np.concatenate([[0], np.cumsum(counts)[:-1]])
    pos = np.arange(batch.shape[0], dtype=np.int64) - offsets[batch]
    dense = np.zeros((B, N, D), np.float32)
    valid = (pos >= 0) & (pos < N) & (batch >= 0) & (batch < B)
    dense[batch[valid], pos[valid]] = x[valid]
    return dense


def _hi_lo(v):
    import ml_dtypes

    hi = v.astype(np.float32).astype(ml_dtypes.bfloat16)
    lo = (v.astype(np.float32) - hi.astype(np.float32)).astype(ml_dtypes.bfloat16)
    return hi, lo


def _make_operands(x, y):
    """x, y: [N, 3] fp32 for one cloud -> (XpT, YpT) [13, N] bf16."""
    import ml_dtypes

    xT = x.T.astype(np.float64)  # [3, N]
    yT = y.T.astype(np.float64)
    x2 = (xT * xT).sum(axis=0)  # [N]
    y2 = (yT * yT).sum(axis=0)
    y2m = -2.0 * yT  # [3, N]

    Xp = np.zeros((K, N), ml_dtypes.bfloat16)
    Yp = np.zeros((K, N), ml_dtypes.bfloat16)
    ones = np.ones((N,), ml_dtypes.bfloat16)
    for i in range(D):
        hx, lx = _hi_lo(xT[i])
        hy, ly = _hi_lo(y2m[i])
        Xp[3 * i + 0], Yp[3 * i + 0] = hx, hy
        Xp[3 * i + 1], Yp[3 * i + 1] = hx, ly
        Xp[3 * i + 2], Yp[3 * i + 2] = lx, hy
    hx2, lx2 = _hi_lo(x2)
    hy2, ly2 = _hi_lo(y2)
    Xp[9], Yp[9] = hx2, ones
    Xp[10], Yp[10] = lx2, ones
    Xp[11], Yp[11] = ones, hy2
    Xp[12], Yp[12] = ones, ly2
    return Xp, Yp


def kernel(pred, target, batch):
    global LAST_EXEC_NS
    from concourse.bass_utils import run_bass_kernel_spmd

    import ml_dtypes

    xd = _to_dense(pred, batch)  # [B, N, 3]
    yd = _to_dense(target, batch)

    RES_W = N + NCHUNK * 256
    in_maps = []
    for core in range(NCORES):
        inp = np.zeros((K, CPC, 2, N), ml_dtypes.bfloat16)
        for c in range(CPC):
            b = core * CPC + c
            Xp, Yp = _make_operands(xd[b], yd[b])
            inp[:, c, 0, :] = Xp
            inp[:, c, 1, :] = Yp
        in_maps.append({"inp": inp})

    if TRACE:
        _install_profile_shim()
    nc = _get_nc()
    res = run_bass_kernel_spmd(
        nc, in_maps, core_ids=list(range(NCORES)), trace=TRACE
    )
    LAST_EXEC_NS = res.exec_time_ns

    total = 0.0
    for core in range(NCORES):
        out = np.asarray(res.results[core]["out"], np.float64)  # [128, CPC*RES_W]
        for c in range(CPC):
            colacc = out[:, c * RES_W : c * RES_W + N]
            rowblk = out[:, c * RES_W + N : (c + 1) * RES_W]
            rowmins = rowblk.reshape(128, NCHUNK, 256).min(axis=2)
            cham_x = rowmins.mean()
            cham_y = colacc.min(axis=0).mean()
            total += cham_x + cham_y
    return np.float32(total / B)
